# revision 1
# baseline (speedup 1.0000x reference)
"""CapsNet dynamic-routing kernel for TRN2, 8 NeuronCores, data-parallel over batch.

Reference computes u_hat = u_vecs @ W ([64,1024,2048], 137 GFLOP) then 3 routing
iterations over it. This kernel never materializes u_hat: every routing
contraction is re-associated through u_vecs / W directly:

  v[b,n,:]   = sum_i c[b,n,i] u_vecs[b,i,:]          (contract i, 1024)
  pre[b,n,:] = v[b,n,:] @ W_n                         (contract k, per capsule)
  outputs    = squash(pre)
  w2[b,n,:]  = outputs[b,n,:] @ W_n^T                 (contract d, per capsule)
  b[b,:,i]   = w2[b,:,:] @ u_vecs[b,i,:]^T            (contract k, 512)

~15x fewer FLOPs than materializing u_hat. fp16 operands / fp32 accumulation.

Per core: 8 batches. Host ships fp16 casts, a transposed copy of u_vecs, and
the iter-0 column sums (softmax(0) is uniform), so no on-chip u_vecs
transposes are needed.
"""

import numpy as np

ROUTINGS = 3
NC_CAP = 32
DC = 64
EPS = 1e-7
N_CORES = 8
B, N_IN, D_IN = 64, 1024, 512
B_LOC = B // N_CORES  # 8

_cached = {}


def _build_program():
    import concourse.bass as bass
    import concourse.tile as tile
    from concourse import bacc, mybir

    f16 = mybir.dt.float16
    f32 = mybir.dt.float32
    ADD = mybir.AluOpType.add
    AX = mybir.AxisListType.X
    AF = mybir.ActivationFunctionType

    nc = bacc.Bacc("TRN2", target_bir_lowering=False, debug=False,
                   num_devices=N_CORES)

    u16_d = nc.dram_tensor("u16", [B_LOC, N_IN, D_IN], f16, kind="ExternalInput").ap()
    ut16_d = nc.dram_tensor("ut16", [B_LOC, D_IN, N_IN], f16, kind="ExternalInput").ap()
    w16_d = nc.dram_tensor("w16", [D_IN, NC_CAP * DC], f16, kind="ExternalInput").ap()
    # WT packed: [128=(tau,d), 16=(m,g), 512] ; capsule n = 8m + 4tau + g
    wt16_d = nc.dram_tensor("wt16", [128, 16, D_IN], f16, kind="ExternalInput").ap()
    # s32T: column sums of u_vecs / 32, transposed: [128=(k%128), 4=(k//128), 8=b']
    s32t_d = nc.dram_tensor("s32t", [128, 4, B_LOC], f16, kind="ExternalInput").ap()
    ident_d = nc.dram_tensor("ident", [128, 128], f16, kind="ExternalInput").ap()
    out_d = nc.dram_tensor("out", [B_LOC, NC_CAP, DC], f32, kind="ExternalOutput").ap()

    with tile.TileContext(nc) as tc:
        with (
            tc.tile_pool(name="big", bufs=1) as big,
            tc.tile_pool(name="work", bufs=1) as work,
            tc.tile_pool(name="ps", bufs=2, space="PSUM") as psp,
            tc.tile_pool(name="ps1", bufs=1, space="PSUM") as psp1,
            tc.tile_pool(name="ps3", bufs=3, space="PSUM") as psp3,
        ):
            U = big.tile([128, B_LOC, 8, D_IN], f16, tag="U")        # (i%128),(b),(i//128),(k)
            UT = big.tile([128, B_LOC, 4, N_IN], f16, tag="UT")      # (k%128),(b),(k//128),(i)
            W16 = big.tile([128, 4, NC_CAP * DC], f16, tag="W16")    # (k%128),(k//128),(n d)
            WT16 = big.tile([128, 16, D_IN], f16, tag="WT16")
            S32T = work.tile([128, 4, B_LOC], f16, tag="S32T")
            IDENT = work.tile([128, 128], f16, tag="IDENT")

            vT_all = work.tile([128, 4, B_LOC, NC_CAP], f16, tag="vT")    # (k%128),(j),(b),(n)
            w2T_all = work.tile([128, 4, B_LOC, NC_CAP], f16, tag="w2T")  # (k%128),(j),(b),(n)
            c_sb = work.tile([128, B_LOC, 8, NC_CAP], f16, tag="c")       # (i%128),(b),(t),(n)
            e_sb = work.tile([128, B_LOC, 8, NC_CAP], f16, tag="e")
            # masked outputs^T for paired w2 matmuls: [(tau d), 16P=(m g), (tau', b)]
            L_sb = work.tile([128, 16, 2, B_LOC], f16, tag="L")
            z_sb = work.tile([128, B_LOC, 8], f32, tag="z")
            r_sb = work.tile([128, B_LOC, 8], f32, tag="r")
            outp16 = work.tile([128, 8, DC], f16, tag="outp16")           # (32g+b),(T),(d)
            outT = work.tile([128, 4, 128], f16, tag="outT")              # (tau d),(m),(32g+b)
            nrm = work.tile([128, 8], f32, tag="nrm")
            sq2 = work.tile([128, 8, DC], f32, tag="sq2")
            sq = work.tile([128, 8], f32, tag="sq")
            scl = work.tile([128, 8], f32, tag="scl")
            outp32 = work.tile([128, 8, DC], f32, tag="outp32")
            eps_t = work.tile([128, 1], f32, tag="eps")
            nc.gpsimd.memset(eps_t[:], EPS)

            # ---- loads ----
            # Single sync ring; multi-ring spreads and per-batch UT/U
            # interleaving both measured slower (128.6-144us vs 127.3us).
            nc.sync.dma_start(W16[:], w16_d.rearrange("(j p) z -> p j z", p=128))
            nc.sync.dma_start(WT16[:], wt16_d[:])
            nc.sync.dma_start(S32T[:], s32t_d[:])
            nc.sync.dma_start(IDENT[:], ident_d[:])
            for b in range(B_LOC):
                nc.sync.dma_start(UT[:, b], ut16_d[b].rearrange("(j p) i -> p j i", p=128))
            for b in range(B_LOC):
                nc.sync.dma_start(U[:, b], u16_d[b].rearrange("(t p) k -> p t k", p=128))

            def caps_mm_pre(pre_ps, lhsT_of):
                # pre[b', n, :]: out rows (g,b') at base 32g, cols (T,d).
                for T in range(8):
                    for g in range(4):
                        for j in range(4):
                            n = 4 * T + g
                            nc.tensor.matmul(
                                pre_ps[32 * g:32 * g + B_LOC, T],
                                lhsT_of(j, n),
                                W16[:, j, n * DC:(n + 1) * DC],
                                start=(j == 0), stop=(j == 3),
                                tile_position=(0, 32 * g),
                            )

            def squash(pre_ps, it):
                nc.scalar.activation(sq2[:], pre_ps[:], AF.Square)
                nc.vector.tensor_reduce(nrm[:], sq2[:], AX, ADD)
                nc.scalar.activation(sq[:], nrm[:], AF.Sqrt, bias=eps_t[:])
                nc.vector.reciprocal(scl[:], sq[:])
                dst = outp16 if it < ROUTINGS - 1 else outp32
                nc.vector.tensor_mul(dst[:], pre_ps[:],
                                     scl[:].broadcast_to((128, 8, DC)))
                if it == ROUTINGS - 1:
                    dr = out_d.rearrange("b (T g) d -> g b T d", g=4)
                    for g in range(4):
                        nc.sync.dma_start(dr[g], outp32[32 * g:32 * g + B_LOC])

            def transpose_and_w2():
                # shares the "pre" slot: tp is only live after pre's readers finish
                tp_ps = psp1.tile([128, 4, 128], f16, tag="pre")
                for m in range(4):
                    nc.tensor.transpose(
                        tp_ps[:, m],
                        outp16[:, 2 * m:2 * m + 2, :].rearrange("p a b -> p (a b)"),
                        IDENT[:])
                nc.vector.tensor_copy(outT[:], tp_ps[:])
                # Build the block-diagonal mask L so one matmul covers a
                # capsule pair: L[(tau,d), P, (tau',b)] = outT iff tau==tau'.
                nc.vector.memset(L_sb[:], 0.0)
                for tau in range(2):
                    nc.vector.tensor_copy(
                        L_sb[64 * tau:64 * tau + 64, :, tau, :],
                        outT[64 * tau:64 * tau + 64, :, :]
                        .rearrange("p m (g c) -> p (m g) c", g=4)[:, :, 0:B_LOC])
                # One matmul per (capsule pair P, k' chunk j): K spans both
                # capsules' d, the zero blocks in L kill cross terms.
                w2pn = psp1.tile([128, 4, 16, 2, B_LOC], f32, tag="w2pn")
                for p in range(16):
                    for j in range(4):
                        nc.tensor.matmul(
                            w2pn[:, j, p],
                            WT16[:, p, 128 * j:128 * j + 128],
                            L_sb[:, p],
                            start=True, stop=True,
                        )
                # w2T_all[:, j, b, n] with n = 8m + 4tau + g
                w2v = w2T_all[:].rearrange("p j b (m x g) -> p x j m g b", m=4, x=2, g=4)
                for tau in range(2):
                    for j in range(4):
                        nc.vector.tensor_copy(
                            w2v[:, tau, j],
                            w2pn[:, j, :, tau].rearrange("p (m g) b -> p m g b", g=4))

            def bupdate_softmax(b):
                b_ps = psp.tile([128, 8, NC_CAP], f32, tag="b_ps")
                for t in range(8):
                    for j in range(4):
                        nc.tensor.matmul(
                            b_ps[:, t], UT[:, b, j, 128 * t:128 * t + 128],
                            w2T_all[:, j, b, :], start=(j == 0), stop=(j == 3))
                nc.scalar.activation(e_sb[:, b], b_ps[:], AF.Exp)
                nc.vector.tensor_reduce(z_sb[:, b], e_sb[:, b], AX, ADD)
                nc.vector.reciprocal(r_sb[:, b], z_sb[:, b])
                nc.vector.tensor_mul(
                    c_sb[:, b], e_sb[:, b],
                    r_sb[:, b].broadcast_to((128, 8, NC_CAP)))

            def v_matmul(b):
                vT_ps = psp3.tile([128, 4, NC_CAP], f32, tag="vT_ps")
                for j in range(4):
                    for t in range(8):
                        nc.tensor.matmul(
                            vT_ps[:, j], U[:, b, t, 128 * j:128 * j + 128],
                            c_sb[:, b, t, :], start=(t == 0), stop=(t == 7))
                nc.scalar.copy(vT_all[:, :, b, :], vT_ps[:])

            # ================= schedule =================
            for it in range(ROUTINGS):
                pre_ps = psp1.tile([128, 8, DC], f32, tag="pre")
                nc.vector.memset(pre_ps[:], 0.0)
                if it == 0:
                    with nc.named_scope(f"i{it}_pre"):
                        caps_mm_pre(pre_ps, lambda j, n: S32T[:, j, :])
                else:
                    with nc.named_scope(f"i{it}_v"):
                        for b in range(B_LOC):
                            v_matmul(b)
                    with nc.named_scope(f"i{it}_pre"):
                        caps_mm_pre(pre_ps, lambda j, n: vT_all[:, j, :, n])
                with nc.named_scope(f"i{it}_squash"):
                    squash(pre_ps, it)
                if it < ROUTINGS - 1:
                    with nc.named_scope(f"i{it}_w2"):
                        transpose_and_w2()
                    with nc.named_scope(f"i{it}_bup"):
                        for b in range(B_LOC):
                            bupdate_softmax(b)

    nc.compile()
    return nc


def _host_prep(u_vecs, W):
    u_vecs = np.asarray(u_vecs, dtype=np.float32)
    W = np.asarray(W, dtype=np.float32).reshape(D_IN, NC_CAP * DC)

    w16 = W.astype(np.float16)
    Wr = W.reshape(D_IN, NC_CAP, DC)  # [k', n, d]
    wt = np.zeros((128, 16, D_IN), dtype=np.float16)
    for m in range(4):
        for g in range(4):
            for tau in range(2):
                n = 8 * m + 4 * tau + g
                wt[64 * tau:64 * tau + 64, 4 * m + g, :] = Wr[:, n, :].T.astype(np.float16)

    ident = np.eye(128, dtype=np.float16)

    in_maps = []
    for c in range(N_CORES):
        ub = u_vecs[c * B_LOC:(c + 1) * B_LOC]  # [8, 1024, 512] fp32
        u16 = ub.astype(np.float16)
        ut16 = np.ascontiguousarray(u16.transpose(0, 2, 1))  # [8, 512, 1024]
        s = ub.sum(axis=1) / NC_CAP                           # [8, 512] fp32
        s32t = np.ascontiguousarray(
            s.T.reshape(4, 128, B_LOC).transpose(1, 0, 2)).astype(np.float16)
        in_maps.append({
            "u16": u16, "ut16": ut16, "w16": w16, "wt16": wt,
            "s32t": s32t, "ident": ident,
        })
    return in_maps


def kernel(u_vecs, W):
    from concourse.bass_utils import run_bass_kernel_spmd

    if "nc" not in _cached:
        _cached["nc"] = _build_program()
    nc = _cached["nc"]

    in_maps = _host_prep(u_vecs, W)
    res = run_bass_kernel_spmd(nc, in_maps, list(range(N_CORES)))
    out = np.concatenate([res.results[c]["out"] for c in range(N_CORES)], axis=0)
    return out.astype(np.float32)



# revision 16
# speedup vs baseline: 1.0214x; 1.0214x over previous
"""CapsNet dynamic-routing kernel for TRN2, 8 NeuronCores, data-parallel over batch.

Routing math is fully batch-local, so the kernel is a per-batch pipeline hidden
under the u-vec DMA stream:

  host: iter-0 (softmax(0) is uniform) -> outputs0, w20 = W @ outputs0 shipped
  chip: per batch b:  b1 = w20 @ u^T -> softmax -> v1 = c1^T u   (as UT/U land)
        per group:    pre1 = v1 @ W -> squash -> w21 = W @ out1  (T-pair pipelined)
        per batch:    b2 = w21 @ u^T -> softmax -> v2
        all batches:  pre2 -> squash -> one output DMA

W^T for the w2 step is built on chip from a capsule-permuted W via TensorE
transposes (saves a 2MB load). fp16 operands / fp32 accumulation.
"""

import numpy as np

ROUTINGS = 3
NC_CAP = 32
DC = 64
EPS = 1e-7
N_CORES = 8
B, N_IN, D_IN = 64, 1024, 512
B_LOC = B // N_CORES  # 8
GA, GB = 4, 4  # iter-1 group sizes (batches 0..3, 4..7)

# capsule n = 8m + 4tau + g lives at storage position 2*(4m+g) + tau in Wp
POS = [0] * NC_CAP
for _m in range(4):
    for _g in range(4):
        for _t in range(2):
            POS[8 * _m + 4 * _t + _g] = 2 * (4 * _m + _g) + _t

_cached = {}


def _build_program():
    import concourse.bass as bass
    import concourse.tile as tile
    from concourse import bacc, mybir

    f16 = mybir.dt.float16
    f32 = mybir.dt.float32
    ADD = mybir.AluOpType.add
    AX = mybir.AxisListType.X
    AF = mybir.ActivationFunctionType

    nc = bacc.Bacc("TRN2", target_bir_lowering=False, debug=False,
                   num_devices=N_CORES)

    # host-packed, SBUF-native layouts (partition dim first, contiguous rows)
    w16_d = nc.dram_tensor("w16", [128, 4, NC_CAP * DC], f16, kind="ExternalInput").ap()
    w20t_d = nc.dram_tensor("w20t", [128, 4, B_LOC, NC_CAP], f16, kind="ExternalInput").ap()
    ut_d = nc.dram_tensor("ut16", [B_LOC, 128, 4, N_IN], f16, kind="ExternalInput").ap()
    u_d = nc.dram_tensor("u16", [B_LOC, 128, 8, D_IN], f16, kind="ExternalInput").ap()
    ident_d = nc.dram_tensor("ident", [128, 128], f16, kind="ExternalInput").ap()
    out_d = nc.dram_tensor("out", [128, 8 * DC], f32, kind="ExternalOutput").ap()

    with tile.TileContext(nc) as tc:
        with (
            tc.tile_pool(name="big", bufs=1) as big,
            tc.tile_pool(name="work", bufs=1) as work,
            tc.tile_pool(name="sbE", bufs=2) as sbE,
            tc.tile_pool(name="psB", bufs=2, space="PSUM") as psB,
            tc.tile_pool(name="psV", bufs=2, space="PSUM") as psV,
            tc.tile_pool(name="psPre", bufs=1, space="PSUM") as psPre,
            tc.tile_pool(name="psT", bufs=2, space="PSUM") as psT,
            tc.tile_pool(name="psW2", bufs=1, space="PSUM") as psW2,
        ):
            U = big.tile([128, B_LOC, 8, D_IN], f16, tag="U")      # (i%128),(b),(i//128),(k)
            UT = big.tile([128, B_LOC, 4, N_IN], f16, tag="UT")    # (k%128),(b),(k//128),(i)
            W16 = big.tile([128, 4, NC_CAP * DC], f16, tag="W16")  # (k%128),(k//128),(pos d)
            WT16 = big.tile([128, 16, D_IN], f16, tag="WT16")      # (tau d),(m g),(k)
            W20T = big.tile([128, 4, B_LOC, NC_CAP], f16, tag="W20T")
            IDENT = work.tile([128, 128], f16, tag="IDENT")

            vT_all = work.tile([128, 4, B_LOC, NC_CAP], f16, tag="vT")
            w2T_all = work.tile([128, 4, B_LOC, NC_CAP], f16, tag="w2T")
            c_sb = work.tile([128, B_LOC, 8, NC_CAP], f16, tag="c")
            z_sb = work.tile([128, B_LOC, 8], f32, tag="z")
            r_sb = work.tile([128, B_LOC, 8], f32, tag="r")
            sq2 = work.tile([128, 2, DC], f32, tag="sq2")
            nrm = work.tile([128, 8], f32, tag="nrm")
            srt = work.tile([128, 8], f32, tag="srt")
            scl = work.tile([128, 8], f32, tag="scl")
            outp16 = [work.tile([128, 8, DC], f16, tag=f"outp16_{g}",
                                name=f"outp16_{g}") for g in range(2)]
            outT = [work.tile([128, 4, 128], f16, tag=f"outT_{g}",
                              name=f"outT_{g}") for g in range(2)]
            L_sb = [work.tile([128, 16, 2, 4], f16, tag=f"L_{g}",
                              name=f"L_{g}") for g in range(2)]
            outp32 = work.tile([128, 8, DC], f32, tag="outp32")
            eps_t = work.tile([128, 1], f32, tag="eps")
            dum = work.tile([128, 2], f32, tag="dum")

            # ---- DMA queue (single sync ring, issue order = arrival order) ----
            nc.sync.dma_start(W16[:], w16_d[:])
            nc.sync.dma_start(IDENT[:], ident_d[:])
            nc.sync.dma_start(W20T[:], w20t_d[:])
            for b in range(B_LOC):
                nc.sync.dma_start(UT[:, b], ut_d[b])
                nc.sync.dma_start(U[:, b], u_d[b])

            # ---- warm activation tables / constants under the DMA shadow ----
            nc.gpsimd.memset(eps_t[:], EPS)
            nc.gpsimd.memset(dum[:], 1.0)
            nc.scalar.activation(dum[:, 0:1], dum[:, 1:2], AF.Exp)
            nc.scalar.activation(dum[:, 0:1], dum[:, 1:2], AF.Sqrt)
            nc.scalar.activation(dum[:, 0:1], dum[:, 1:2], AF.Square)
            nc.scalar.copy(dum[:, 0:1], dum[:, 1:2])
            nc.vector.memset(L_sb[0][:], 0.0)
            nc.vector.memset(L_sb[1][:], 0.0)

            # ---- build WT16 on chip: transpose capsule-pair blocks of W16 ----
            with nc.named_scope("wt_build"):
                for t in range(16):
                    wtt = psT.tile([128, 4, 128], f16, tag="tp")
                    for j in range(4):
                        nc.tensor.transpose(
                            wtt[:, j], W16[:, j, 128 * t:128 * t + 128], IDENT[:])
                    nc.vector.tensor_copy(
                        WT16[:, t], wtt[:].rearrange("p a b -> p (a b)"))

            def bup(b, src, it):
                # b-logits for batch b: [i%128, t, n] = sum_k u^T chunks @ w2T
                with nc.named_scope(f"i{it}_bup{b}"):
                    b_ps = psB.tile([128, 8, NC_CAP], f32, tag="b_ps")
                    for t in range(8):
                        for j in range(4):
                            nc.tensor.matmul(
                                b_ps[:, t], UT[:, b, j, 128 * t:128 * t + 128],
                                src[:, j, b, :], start=(j == 0), stop=(j == 3))
                    e_sb = sbE.tile([128, 8, NC_CAP], f16, tag="e_sb")
                    nc.scalar.activation(e_sb[:], b_ps[:], AF.Exp)
                    nc.vector.tensor_reduce(z_sb[:, b], e_sb[:], AX, ADD)
                    nc.vector.reciprocal(r_sb[:, b], z_sb[:, b])
                    nc.vector.tensor_mul(
                        c_sb[:, b], e_sb[:],
                        r_sb[:, b].broadcast_to((128, 8, NC_CAP)))

            def vmm(b, it):
                with nc.named_scope(f"i{it}_v{b}"):
                    vT_ps = psV.tile([128, 4, NC_CAP], f32, tag="vT_ps")
                    for j in range(4):
                        for t in range(8):
                            nc.tensor.matmul(
                                vT_ps[:, j], U[:, b, t, 128 * j:128 * j + 128],
                                c_sb[:, b, t, :], start=(t == 0), stop=(t == 7))
                    nc.scalar.copy(vT_all[:, :, b, :], vT_ps[:])

            def pre_piece(pre_ps, b0, nb, m):
                # capsules n = 4T+g for T in {2m, 2m+1}
                for T in (2 * m, 2 * m + 1):
                    for g in range(4):
                        n = 4 * T + g
                        for j in range(4):
                            nc.tensor.matmul(
                                pre_ps[32 * g:32 * g + nb, T],
                                vT_all[:, j, b0:b0 + nb, n],
                                W16[:, j, 64 * POS[n]:64 * POS[n] + 64],
                                start=(j == 0), stop=(j == 3),
                                tile_position=(0, 32 * g),
                            )

            def squash_piece(pre_ps, dst, m):
                sl = slice(2 * m, 2 * m + 2)
                nc.scalar.activation(sq2[:], pre_ps[:, sl], AF.Square)
                nc.vector.tensor_reduce(nrm[:, sl], sq2[:], AX, ADD)
                nc.scalar.activation(srt[:, sl], nrm[:, sl], AF.Sqrt, bias=eps_t[:])
                nc.vector.reciprocal(scl[:, sl], srt[:, sl])
                nc.vector.tensor_mul(
                    dst[:, sl], pre_ps[:, sl],
                    scl[:, sl].broadcast_to((128, 2, DC)))

            def w2_tp_piece(grp, m):
                # transpose scaled outputs T-pair m -> outT[(tau d), m, (g c)]
                tp = psT.tile([128, 4, 128], f16, tag="tp")
                nc.tensor.transpose(
                    tp[:, 0], outp16[grp][:, 2 * m:2 * m + 2, :]
                    .rearrange("p a b -> p (a b)"), IDENT[:])
                nc.vector.tensor_copy(outT[grp][:, m], tp[:, 0])
                for tau in range(2):
                    nc.vector.tensor_copy(
                        L_sb[grp][64 * tau:64 * tau + 64, 4 * m:4 * m + 4, tau, :],
                        outT[grp][64 * tau:64 * tau + 64, m, :]
                        .rearrange("p (g c) -> p g c", g=4)[:, :, 0:4])

            def w2_mm_piece(grp, w2pn, m):
                for p in range(4 * m, 4 * m + 4):
                    for j in range(4):
                        nc.tensor.matmul(
                            w2pn[:, j, p], WT16[:, p, 128 * j:128 * j + 128],
                            L_sb[grp][:, p], start=True, stop=True)

            def w2_gather(grp, w2pn, b0):
                w2v = w2T_all[:].rearrange(
                    "p j b (m x g) -> p x j m g b", m=4, x=2, g=4)
                for tau in range(2):
                    for j in range(4):
                        nc.vector.tensor_copy(
                            w2v[:, tau, j, :, :, b0:b0 + 4],
                            w2pn[:, j, :, tau].rearrange(
                                "p (m g) b -> p m g b", g=4))

            def pre1_squash_w2(grp, b0):
                # pre -> squash -> w2 for a 4-batch group, pipelined by T-pair
                with nc.named_scope(f"g{grp}_pre1w2"):
                    pre_ps = psPre.tile([128, 8, DC], f32, tag="pre")
                    w2pn = psW2.tile([128, 4, 16, 2, 4], f32, tag="w2pn")
                    pre_piece(pre_ps, b0, 4, 0)
                    squash_piece(pre_ps, outp16[grp], 0)
                    pre_piece(pre_ps, b0, 4, 1)
                    squash_piece(pre_ps, outp16[grp], 1)
                    w2_tp_piece(grp, 0)
                    w2_mm_piece(grp, w2pn, 0)
                    pre_piece(pre_ps, b0, 4, 2)
                    squash_piece(pre_ps, outp16[grp], 2)
                    w2_tp_piece(grp, 1)
                    w2_mm_piece(grp, w2pn, 1)
                    pre_piece(pre_ps, b0, 4, 3)
                    squash_piece(pre_ps, outp16[grp], 3)
                    w2_tp_piece(grp, 2)
                    w2_mm_piece(grp, w2pn, 2)
                    w2_tp_piece(grp, 3)
                    w2_mm_piece(grp, w2pn, 3)
                    w2_gather(grp, w2pn, b0)

            # ================= schedule =================
            # group A iter-1 (batches 0..3 as their tiles land)
            for b in range(4):
                bup(b, W20T, 1)
                vmm(b, 1)
            pre1_squash_w2(0, 0)

            # interleave group B iter-1 with group A iter-2
            bup(4, W20T, 1); vmm(4, 1)
            bup(5, W20T, 1); vmm(5, 1)
            bup(0, w2T_all, 2); vmm(0, 2)
            bup(6, W20T, 1); vmm(6, 1)
            bup(1, w2T_all, 2); vmm(1, 2)
            bup(2, w2T_all, 2); vmm(2, 2)
            bup(7, W20T, 1); vmm(7, 1)
            bup(3, w2T_all, 2); vmm(3, 2)

            # group B pre1 -> squash -> w2 (tail-critical, T-pair pipelined)
            pre1_squash_w2(1, 4)

            # group B iter-2
            for b in range(4, B_LOC):
                bup(b, w2T_all, 2)
                vmm(b, 2)

            # final pre over all 8 batches, squash, single output DMA
            with nc.named_scope("pre2_out"):
                pre2_ps = psPre.tile([128, 8, DC], f32, tag="pre")
                for m in range(4):
                    pre_piece(pre2_ps, 0, 8, m)
                    squash_piece(pre2_ps, outp32, m)
                nc.sync.dma_start(
                    out_d[:], outp32[:].rearrange("p a b -> p (a b)"))

    nc.compile()
    return nc


def _host_prep(u_vecs, W):
    u_vecs = np.asarray(u_vecs, dtype=np.float32)
    W = np.asarray(W, dtype=np.float32).reshape(D_IN, NC_CAP * DC)
    Wr = W.reshape(D_IN, NC_CAP, DC)

    # capsule-permuted W so on-chip transposes of 128-col blocks give tau-pairs
    perm = np.argsort(POS)  # perm[pos] = capsule n stored at pos
    Wp = np.ascontiguousarray(Wr[:, perm, :]).reshape(D_IN, NC_CAP * DC)
    w16 = np.ascontiguousarray(
        Wp.reshape(4, 128, NC_CAP * DC).transpose(1, 0, 2)).astype(np.float16)
    ident = np.eye(128, dtype=np.float16)

    in_maps = []
    for c in range(N_CORES):
        ub = u_vecs[c * B_LOC:(c + 1) * B_LOC]  # [8, 1024, 512] fp32
        u16 = ub.astype(np.float16)
        up = np.ascontiguousarray(
            u16.reshape(B_LOC, 8, 128, D_IN).transpose(0, 2, 1, 3))
        utp = np.ascontiguousarray(
            u16.transpose(0, 2, 1).reshape(B_LOC, 4, 128, N_IN)
            .transpose(0, 2, 1, 3))
        # host iter-0: c is uniform, so outputs0 depends only on column sums
        s = ub.sum(axis=1) / NC_CAP                       # [8, 512] fp32
        pre0 = np.einsum('bk,knd->bnd', s, Wr)
        out0 = pre0 / np.sqrt((pre0 ** 2).sum(-1, keepdims=True) + EPS)
        w20 = np.einsum('bnd,knd->bnk', out0, Wr)         # [8, 32, 512]
        w20t = np.ascontiguousarray(
            w20.transpose(2, 0, 1).reshape(4, 128, B_LOC, NC_CAP)
            .transpose(1, 0, 2, 3)).astype(np.float16)
        in_maps.append({
            "u16": up, "ut16": utp, "w16": w16, "w20t": w20t, "ident": ident,
        })
    return in_maps


def _unpack_out(raw):
    # raw [128, 512] f32; row 32g+b, cols (T, d) -> out[b, 4T+g, d]
    r = raw.reshape(4, 32, 8, DC)     # [g, b-slot, T, d]
    out = np.empty((B_LOC, NC_CAP, DC), dtype=np.float32)
    for g in range(4):
        for b in range(B_LOC):
            out[b, 4 * np.arange(8) + g, :] = r[g, b]
    return out


def kernel(u_vecs, W):
    from concourse.bass_utils import run_bass_kernel_spmd

    if "nc" not in _cached:
        _cached["nc"] = _build_program()
    nc = _cached["nc"]

    in_maps = _host_prep(u_vecs, W)
    res = run_bass_kernel_spmd(nc, in_maps, list(range(N_CORES)))
    out = np.concatenate(
        [_unpack_out(res.results[c]["out"]) for c in range(N_CORES)], axis=0)
    return out.astype(np.float32)


# revision 21
# speedup vs baseline: 1.0335x; 1.0119x over previous
"""CapsNet dynamic-routing kernel for TRN2, 8 NeuronCores, data-parallel over batch.

Routing math is fully batch-local, so the kernel is a per-batch pipeline hidden
under the u-vec DMA stream:

  host: iter-0 (softmax(0) is uniform) -> outputs0, w20 = W @ outputs0 shipped
  chip: per batch b:  b1 = w20 @ u^T -> softmax -> v1 = c1^T u   (as UT/U land)
        per group:    pre1 = v1 @ W -> squash -> w21 = W @ out1  (T-pair pipelined)
        per batch:    b2 = w21 @ u^T -> softmax -> v2
        all batches:  pre2 -> squash -> one output DMA

fp16 operands / fp32 accumulation; all inputs host-packed in SBUF-native
partition-major layouts for maximum DMA descriptor size.
"""

import numpy as np

ROUTINGS = 3
NC_CAP = 32
DC = 64
EPS = 1e-7
N_CORES = 8
B, N_IN, D_IN = 64, 1024, 512
B_LOC = B // N_CORES  # 8
GA, GB = 4, 4  # iter-1 group sizes (batches 0..3, 4..7)

_cached = {}


def _build_program():
    import concourse.bass as bass
    import concourse.tile as tile
    from concourse import bacc, mybir

    f16 = mybir.dt.float16
    f32 = mybir.dt.float32
    ADD = mybir.AluOpType.add
    AX = mybir.AxisListType.X
    AF = mybir.ActivationFunctionType

    nc = bacc.Bacc("TRN2", target_bir_lowering=False, debug=False,
                   num_devices=N_CORES)

    # host-packed, SBUF-native layouts (partition dim first, contiguous rows)
    w16_d = nc.dram_tensor("w16", [128, 4, NC_CAP * DC], f16, kind="ExternalInput").ap()
    wt16_d = nc.dram_tensor("wt16", [128, 16, D_IN], f16, kind="ExternalInput").ap()
    w20t_d = nc.dram_tensor("w20t", [128, 4, B_LOC, NC_CAP], f16, kind="ExternalInput").ap()
    ut_d = nc.dram_tensor("ut16", [B_LOC, 128, 4, N_IN], f16, kind="ExternalInput").ap()
    u_d = nc.dram_tensor("u16", [B_LOC, 128, 8, D_IN], f16, kind="ExternalInput").ap()
    ident_d = nc.dram_tensor("ident", [128, 128], f16, kind="ExternalInput").ap()
    out_d = nc.dram_tensor("out", [128, 8 * DC], f32, kind="ExternalOutput").ap()

    with tile.TileContext(nc) as tc:
        with (
            tc.tile_pool(name="big", bufs=1) as big,
            tc.tile_pool(name="work", bufs=1) as work,
            tc.tile_pool(name="sbE", bufs=2) as sbE,
            tc.tile_pool(name="psB", bufs=2, space="PSUM") as psB,
            tc.tile_pool(name="psV", bufs=2, space="PSUM") as psV,
            tc.tile_pool(name="psPre", bufs=1, space="PSUM") as psPre,
            tc.tile_pool(name="psT", bufs=2, space="PSUM") as psT,
            tc.tile_pool(name="psW2", bufs=1, space="PSUM") as psW2,
        ):
            U = big.tile([128, B_LOC, 8, D_IN], f16, tag="U")      # (i%128),(b),(i//128),(k)
            UT = big.tile([128, B_LOC, 4, N_IN], f16, tag="UT")    # (k%128),(b),(k//128),(i)
            W16 = big.tile([128, 4, NC_CAP * DC], f16, tag="W16")  # (k%128),(k//128),(pos d)
            WT16 = big.tile([128, 16, D_IN], f16, tag="WT16")      # (tau d),(m g),(k)
            W20T = big.tile([128, 4, B_LOC, NC_CAP], f16, tag="W20T")
            IDENT = work.tile([128, 128], f16, tag="IDENT")

            vT_all = work.tile([128, 4, B_LOC, NC_CAP], f16, tag="vT")
            w2T_all = work.tile([128, 4, B_LOC, NC_CAP], f16, tag="w2T")
            c_sb = work.tile([128, B_LOC, 8, NC_CAP], f16, tag="c")
            z_sb = work.tile([128, B_LOC, 8], f32, tag="z")
            r_sb = work.tile([128, B_LOC, 8], f32, tag="r")
            sq2 = work.tile([128, 2, DC], f32, tag="sq2")
            pre_c = work.tile([128, 2, DC], f32, tag="pre_c")
            nrm = work.tile([128, 8], f32, tag="nrm")
            srt = work.tile([128, 8], f32, tag="srt")
            scl = work.tile([128, 8], f32, tag="scl")
            outp16 = [work.tile([128, 8, DC], f16, tag=f"outp16_{g}",
                                name=f"outp16_{g}") for g in range(2)]
            outT = [work.tile([128, 4, 128], f16, tag=f"outT_{g}",
                              name=f"outT_{g}") for g in range(2)]
            L_sb = [work.tile([128, 16, 2, 4], f16, tag=f"L_{g}",
                              name=f"L_{g}") for g in range(2)]
            outp32 = work.tile([128, 8, DC], f32, tag="outp32")
            eps_t = work.tile([128, 1], f32, tag="eps")
            dum = work.tile([128, 2], f32, tag="dum")

            # ---- DMA queue (single sync ring, issue order = arrival order) ----
            nc.sync.dma_start(IDENT[:], ident_d[:])
            nc.sync.dma_start(W20T[:], w20t_d[:])
            for b in range(2):
                nc.sync.dma_start(UT[:, b], ut_d[b])
                nc.sync.dma_start(U[:, b], u_d[b])
            nc.sync.dma_start(W16[:], w16_d[:])
            for b in range(2, 4):
                nc.sync.dma_start(UT[:, b], ut_d[b])
                nc.sync.dma_start(U[:, b], u_d[b])
            nc.sync.dma_start(WT16[:], wt16_d[:])
            for b in range(4, B_LOC):
                nc.sync.dma_start(UT[:, b], ut_d[b])
                nc.sync.dma_start(U[:, b], u_d[b])

            # ---- warm activation tables / constants under the DMA shadow ----
            nc.gpsimd.memset(eps_t[:], EPS)
            nc.gpsimd.memset(dum[:], 1.0)
            nc.scalar.activation(dum[:, 0:1], dum[:, 1:2], AF.Exp)
            nc.scalar.activation(dum[:, 0:1], dum[:, 1:2], AF.Sqrt)
            nc.scalar.copy(dum[:, 0:1], dum[:, 1:2])
            nc.vector.memset(L_sb[0][:], 0.0)
            nc.vector.memset(L_sb[1][:], 0.0)

            def bup(b, src, it):
                # b-logits for batch b: [i%128, t, n] = sum_k u^T chunks @ w2T
                with nc.named_scope(f"i{it}_bup{b}"):
                    b_ps = psB.tile([128, 8, NC_CAP], f32, tag="b_ps")
                    for t in range(8):
                        for j in range(4):
                            nc.tensor.matmul(
                                b_ps[:, t], UT[:, b, j, 128 * t:128 * t + 128],
                                src[:, j, b, :], start=(j == 0), stop=(j == 3))
                    e_sb = sbE.tile([128, 8, NC_CAP], f16, tag="e_sb")
                    nc.scalar.activation(e_sb[:], b_ps[:], AF.Exp)
                    nc.vector.tensor_reduce(z_sb[:, b], e_sb[:], AX, ADD)
                    nc.vector.reciprocal(r_sb[:, b], z_sb[:, b])
                    nc.vector.tensor_mul(
                        c_sb[:, b], e_sb[:],
                        r_sb[:, b].broadcast_to((128, 8, NC_CAP)))

            def vmm(b, it):
                with nc.named_scope(f"i{it}_v{b}"):
                    vT_ps = psV.tile([128, 4, NC_CAP], f32, tag="vT_ps")
                    for j in range(4):
                        for t in range(8):
                            nc.tensor.matmul(
                                vT_ps[:, j], U[:, b, t, 128 * j:128 * j + 128],
                                c_sb[:, b, t, :], start=(t == 0), stop=(t == 7))
                    nc.scalar.copy(vT_all[:, :, b, :], vT_ps[:])

            def pre_piece(pre_ps, b0, nb, m):
                # capsules n = 4T+g for T in {2m, 2m+1}
                for T in (2 * m, 2 * m + 1):
                    for g in range(4):
                        n = 4 * T + g
                        for j in range(4):
                            nc.tensor.matmul(
                                pre_ps[32 * g:32 * g + nb, T],
                                vT_all[:, j, b0:b0 + nb, n],
                                W16[:, j, 64 * n:64 * n + 64],
                                start=(j == 0), stop=(j == 3),
                                tile_position=(0, 32 * g),
                            )

            def squash_piece(pre_ps, dst, m):
                sl = slice(2 * m, 2 * m + 2)
                nc.vector.tensor_copy(pre_c[:], pre_ps[:, sl])
                nc.vector.tensor_mul(sq2[:], pre_c[:], pre_ps[:, sl])
                nc.vector.tensor_reduce(nrm[:, sl], sq2[:], AX, ADD)
                nc.scalar.activation(srt[:, sl], nrm[:, sl], AF.Sqrt, bias=eps_t[:])
                nc.vector.reciprocal(scl[:, sl], srt[:, sl])
                nc.vector.tensor_mul(
                    dst[:, sl], pre_ps[:, sl],
                    scl[:, sl].broadcast_to((128, 2, DC)))

            def w2_tp_piece(grp, m):
                # transpose scaled outputs T-pair m -> outT[(tau d), m, (g c)]
                tp = psT.tile([128, 4, 128], f16, tag="tp")
                nc.tensor.transpose(
                    tp[:, 0], outp16[grp][:, 2 * m:2 * m + 2, :]
                    .rearrange("p a b -> p (a b)"), IDENT[:])
                nc.vector.tensor_copy(outT[grp][:, m], tp[:, 0])
                for tau in range(2):
                    nc.vector.tensor_copy(
                        L_sb[grp][64 * tau:64 * tau + 64, 4 * m:4 * m + 4, tau, :],
                        outT[grp][64 * tau:64 * tau + 64, m, :]
                        .rearrange("p (g c) -> p g c", g=4)[:, :, 0:4])

            def w2_mm_piece(grp, w2pn, m):
                for p in range(4 * m, 4 * m + 4):
                    for j in range(4):
                        nc.tensor.matmul(
                            w2pn[:, j, p], WT16[:, p, 128 * j:128 * j + 128],
                            L_sb[grp][:, p], start=True, stop=True)

            def w2_gather(grp, w2pn, b0):
                w2v = w2T_all[:].rearrange(
                    "p j b (m x g) -> p x j m g b", m=4, x=2, g=4)
                for tau in range(2):
                    for j in range(4):
                        nc.vector.tensor_copy(
                            w2v[:, tau, j, :, :, b0:b0 + 4],
                            w2pn[:, j, :, tau].rearrange(
                                "p (m g) b -> p m g b", g=4))

            def pre1_squash_w2(grp, b0):
                # pre -> squash -> w2 for a 4-batch group, pipelined by T-pair
                with nc.named_scope(f"g{grp}_pre1w2"):
                    pre_ps = psPre.tile([128, 8, DC], f32, tag="pre")
                    w2pn = psW2.tile([128, 4, 16, 2, 4], f32, tag="w2pn")
                    pre_piece(pre_ps, b0, 4, 0)
                    squash_piece(pre_ps, outp16[grp], 0)
                    pre_piece(pre_ps, b0, 4, 1)
                    squash_piece(pre_ps, outp16[grp], 1)
                    w2_tp_piece(grp, 0)
                    w2_mm_piece(grp, w2pn, 0)
                    pre_piece(pre_ps, b0, 4, 2)
                    squash_piece(pre_ps, outp16[grp], 2)
                    w2_tp_piece(grp, 1)
                    w2_mm_piece(grp, w2pn, 1)
                    pre_piece(pre_ps, b0, 4, 3)
                    squash_piece(pre_ps, outp16[grp], 3)
                    w2_tp_piece(grp, 2)
                    w2_mm_piece(grp, w2pn, 2)
                    w2_tp_piece(grp, 3)
                    w2_mm_piece(grp, w2pn, 3)
                    w2_gather(grp, w2pn, b0)

            # ================= schedule =================
            # pair-interleaved so v(b) never stalls Tensor on softmax latency
            def pair1(a, b):
                bup(a, W20T, 1); bup(b, W20T, 1); vmm(a, 1); vmm(b, 1)

            def pair2(a, b):
                bup(a, w2T_all, 2); bup(b, w2T_all, 2); vmm(a, 2); vmm(b, 2)

            pair1(0, 1)
            pair1(2, 3)
            pre1_squash_w2(0, 0)
            pair1(4, 5)
            pair2(0, 1)
            pair1(6, 7)
            pair2(2, 3)
            pre1_squash_w2(1, 4)
            pair2(4, 5)
            pair2(6, 7)

            # final pre over all 8 batches, squash, single output DMA
            with nc.named_scope("pre2_out"):
                pre2_ps = psPre.tile([128, 8, DC], f32, tag="pre")
                for m in range(4):
                    pre_piece(pre2_ps, 0, 8, m)
                    squash_piece(pre2_ps, outp32, m)
                nc.sync.dma_start(
                    out_d[:], outp32[:].rearrange("p a b -> p (a b)"))

    nc.compile()
    return nc


def _host_prep(u_vecs, W):
    u_vecs = np.asarray(u_vecs, dtype=np.float32)
    W = np.asarray(W, dtype=np.float32).reshape(D_IN, NC_CAP * DC)
    Wr = W.reshape(D_IN, NC_CAP, DC)

    w16 = np.ascontiguousarray(
        W.reshape(4, 128, NC_CAP * DC).transpose(1, 0, 2)).astype(np.float16)
    # WT packed: [128=(tau,d), 16=(m,g), 512]; capsule n = 8m + 4tau + g
    wt = np.zeros((128, 16, D_IN), dtype=np.float16)
    for m in range(4):
        for g in range(4):
            for tau in range(2):
                n = 8 * m + 4 * tau + g
                wt[64 * tau:64 * tau + 64, 4 * m + g, :] = \
                    Wr[:, n, :].T.astype(np.float16)
    ident = np.eye(128, dtype=np.float16)

    in_maps = []
    for c in range(N_CORES):
        ub = u_vecs[c * B_LOC:(c + 1) * B_LOC]  # [8, 1024, 512] fp32
        u16 = ub.astype(np.float16)
        up = np.ascontiguousarray(
            u16.reshape(B_LOC, 8, 128, D_IN).transpose(0, 2, 1, 3))
        utp = np.ascontiguousarray(
            u16.transpose(0, 2, 1).reshape(B_LOC, 4, 128, N_IN)
            .transpose(0, 2, 1, 3))
        # host iter-0: c is uniform, so outputs0 depends only on column sums
        s = ub.sum(axis=1) / NC_CAP                       # [8, 512] fp32
        pre0 = np.einsum('bk,knd->bnd', s, Wr)
        out0 = pre0 / np.sqrt((pre0 ** 2).sum(-1, keepdims=True) + EPS)
        w20 = np.einsum('bnd,knd->bnk', out0, Wr)         # [8, 32, 512]
        w20t = np.ascontiguousarray(
            w20.transpose(2, 0, 1).reshape(4, 128, B_LOC, NC_CAP)
            .transpose(1, 0, 2, 3)).astype(np.float16)
        in_maps.append({
            "u16": up, "ut16": utp, "w16": w16, "wt16": wt, "w20t": w20t,
            "ident": ident,
        })
    return in_maps


def _unpack_out(raw):
    # raw [128, 512] f32; row 32g+b, cols (T, d) -> out[b, 4T+g, d]
    r = raw.reshape(4, 32, 8, DC)     # [g, b-slot, T, d]
    out = np.empty((B_LOC, NC_CAP, DC), dtype=np.float32)
    for g in range(4):
        for b in range(B_LOC):
            out[b, 4 * np.arange(8) + g, :] = r[g, b]
    return out


def kernel(u_vecs, W):
    from concourse.bass_utils import run_bass_kernel_spmd

    if "nc" not in _cached:
        _cached["nc"] = _build_program()
    nc = _cached["nc"]

    in_maps = _host_prep(u_vecs, W)
    res = run_bass_kernel_spmd(nc, in_maps, list(range(N_CORES)))
    out = np.concatenate(
        [_unpack_out(res.results[c]["out"]) for c in range(N_CORES)], axis=0)
    return out.astype(np.float32)


# revision 23
# speedup vs baseline: 1.0606x; 1.0262x over previous
"""CapsNet dynamic-routing kernel for TRN2, 8 NeuronCores, data-parallel over batch.

Routing math is fully batch-local, so the kernel is a per-batch pipeline hidden
under the u-vec DMA stream:

  host: iter-0 (softmax(0) is uniform) -> outputs0, w20 = W @ outputs0 shipped
  chip: per batch b:  b1 = w20 @ u^T -> softmax -> v1 = c1^T u   (as UT/U land)
        per group:    pre1 = v1 @ W -> squash -> w21 = W @ out1  (T-pair pipelined)
        per batch:    b2 = w21 @ u^T -> softmax -> v2
        per group:    pre2 -> squash -> output DMA

All tiles that different pipeline stages touch are separate (per group /
per T-pair piece) so the tile framework's WAR tracking never serializes
independent stages. Scalar runs only Exp/Sqrt; table swaps are prefetched
with dummy activations at each boundary (the act table holds one function).
fp16 operands / fp32 accumulation; inputs host-packed partition-major.
"""

import numpy as np

ROUTINGS = 3
NC_CAP = 32
DC = 64
EPS = 1e-7
N_CORES = 8
B, N_IN, D_IN = 64, 1024, 512
B_LOC = B // N_CORES  # 8

_cached = {}


def _build_program():
    import concourse.bass as bass
    import concourse.tile as tile
    from concourse import bacc, mybir

    f16 = mybir.dt.float16
    f32 = mybir.dt.float32
    ADD = mybir.AluOpType.add
    AX = mybir.AxisListType.X
    AF = mybir.ActivationFunctionType

    nc = bacc.Bacc("TRN2", target_bir_lowering=False, debug=False,
                   num_devices=N_CORES)

    # host-packed, SBUF-native layouts (partition dim first, contiguous rows)
    w16_d = nc.dram_tensor("w16", [128, 4, NC_CAP * DC], f16, kind="ExternalInput").ap()
    wt16_d = nc.dram_tensor("wt16", [128, 16, D_IN], f16, kind="ExternalInput").ap()
    w20t_d = nc.dram_tensor("w20t", [128, 4, B_LOC, NC_CAP], f16, kind="ExternalInput").ap()
    ut_d = nc.dram_tensor("ut16", [B_LOC, 128, 4, N_IN], f16, kind="ExternalInput").ap()
    u_d = nc.dram_tensor("u16", [B_LOC, 128, 8, D_IN], f16, kind="ExternalInput").ap()
    ident_d = nc.dram_tensor("ident", [128, 128], f16, kind="ExternalInput").ap()
    outA_d = nc.dram_tensor("outA", [128, 8 * DC], f32, kind="ExternalOutput").ap()
    outB_d = nc.dram_tensor("outB", [128, 8 * DC], f32, kind="ExternalOutput").ap()
    out_drams = [outA_d, outB_d]

    with tile.TileContext(nc) as tc:
        with (
            tc.tile_pool(name="big", bufs=1) as big,
            tc.tile_pool(name="work", bufs=1) as work,
            tc.tile_pool(name="sbE", bufs=2) as sbE,
            tc.tile_pool(name="sbP", bufs=2) as sbP,
            tc.tile_pool(name="sbO", bufs=4) as sbO,
            tc.tile_pool(name="psB", bufs=2, space="PSUM") as psB,
            tc.tile_pool(name="psV", bufs=2, space="PSUM") as psV,
            tc.tile_pool(name="psPre", bufs=2, space="PSUM") as psPre,
            tc.tile_pool(name="psT", bufs=1, space="PSUM") as psT,
            tc.tile_pool(name="psW2", bufs=1, space="PSUM") as psW2,
        ):
            U = big.tile([128, B_LOC, 8, D_IN], f16, tag="U")      # (i%128),(b),(i//128),(k)
            UT = big.tile([128, B_LOC, 4, N_IN], f16, tag="UT")    # (k%128),(b),(k//128),(i)
            W16 = big.tile([128, 4, NC_CAP * DC], f16, tag="W16")  # (k%128),(k//128),(n d)
            WT16 = big.tile([128, 16, D_IN], f16, tag="WT16")      # (tau d),(m g),(k)
            W20T = big.tile([128, 4, B_LOC, NC_CAP], f16, tag="W20T")
            IDENT = work.tile([128, 128], f16, tag="IDENT")

            # per-group tiles (A: batches 0-3, B: 4-7) to avoid false WARs
            vT = [work.tile([128, 4, 4, NC_CAP], f16, tag=f"vT{g}",
                            name=f"vT{g}") for g in range(2)]
            w2T = [work.tile([128, 4, 4, NC_CAP], f16, tag=f"w2T{g}",
                             name=f"w2T{g}") for g in range(2)]
            c_sb = [work.tile([128, 4, 8, NC_CAP], f16, tag=f"c{g}",
                              name=f"c{g}") for g in range(2)]
            outT = [work.tile([128, 4, 128], f16, tag=f"outT{g}",
                              name=f"outT{g}") for g in range(2)]
            L_sb = [[work.tile([128, 4, 2, 4], f16, tag=f"L{g}_{m}",
                               name=f"L{g}_{m}") for m in range(4)]
                    for g in range(2)]
            outp32 = [work.tile([128, 8, DC], f32, tag=f"outp32_{g}",
                                name=f"outp32_{g}") for g in range(2)]
            z_sb = work.tile([128, B_LOC, 8], f32, tag="z")
            r_sb = work.tile([128, B_LOC, 8], f32, tag="r")
            eps_t = work.tile([128, 1], f32, tag="eps")
            dum = work.tile([128, 2], f32, tag="dum")

            # ---- DMA queue; first pairs chunked so batch 0/1 complete first
            nc.sync.dma_start(IDENT[:], ident_d[:])
            nc.sync.dma_start(W20T[:], w20t_d[:])
            for j in range(4):
                nc.sync.dma_start(UT[:, 0, j], ut_d[0, :, j])
            for t in range(4):
                nc.sync.dma_start(U[:, 0, 2 * t:2 * t + 2], u_d[0, :, 2 * t:2 * t + 2])
            for j in range(0, 4, 2):
                nc.sync.dma_start(UT[:, 1, j:j + 2], ut_d[1, :, j:j + 2])
            for t in range(0, 8, 4):
                nc.sync.dma_start(U[:, 1, t:t + 4], u_d[1, :, t:t + 4])
            nc.sync.dma_start(W16[:], w16_d[:])
            for b in range(2, 4):
                nc.sync.dma_start(UT[:, b], ut_d[b])
                nc.sync.dma_start(U[:, b], u_d[b])
            nc.sync.dma_start(WT16[:], wt16_d[:])
            for b in range(4, B_LOC):
                nc.sync.dma_start(UT[:, b], ut_d[b])
                nc.sync.dma_start(U[:, b], u_d[b])

            # ---- constants + Exp table warm (single-slot table: Exp only)
            nc.gpsimd.memset(eps_t[:], EPS)
            nc.gpsimd.memset(dum[:], 1.0)
            nc.scalar.activation(dum[:, 0:1], dum[:, 1:2], AF.Exp)
            for g in range(2):
                for m in range(4):
                    nc.gpsimd.memset(L_sb[g][m][:], 0.0)

            def preload(func, tag):
                # dummy activation: prefetches the act table while idle
                with nc.named_scope(f"preload_{tag}"):
                    nc.scalar.activation(dum[:, 0:1], dum[:, 1:2], func)

            def bup(b, it):
                # b-logits for batch b: [i%128, t, n] = sum_k u^T chunks @ w2T
                src = W20T if it == 1 else w2T[b // 4]
                bl = b if it == 1 else b % 4
                with nc.named_scope(f"i{it}_bup{b}"):
                    b_ps = psB.tile([128, 8, NC_CAP], f32, tag="b_ps")
                    for t in range(8):
                        for j in range(4):
                            nc.tensor.matmul(
                                b_ps[:, t], UT[:, b, j, 128 * t:128 * t + 128],
                                src[:, j, bl, :], start=(j == 0), stop=(j == 3))
                    e_sb = sbE.tile([128, 8, NC_CAP], f16, tag="e_sb")
                    nc.scalar.activation(e_sb[:], b_ps[:], AF.Exp)
                    nc.vector.tensor_reduce(z_sb[:, b], e_sb[:], AX, ADD)
                    nc.vector.reciprocal(r_sb[:, b], z_sb[:, b])
                    nc.vector.tensor_mul(
                        c_sb[b // 4][:, b % 4], e_sb[:],
                        r_sb[:, b].broadcast_to((128, 8, NC_CAP)))

            def vmm(b, it):
                with nc.named_scope(f"i{it}_v{b}"):
                    vT_ps = psV.tile([128, 4, NC_CAP], f32, tag="vT_ps")
                    for j in range(4):
                        for t in range(8):
                            nc.tensor.matmul(
                                vT_ps[:, j], U[:, b, t, 128 * j:128 * j + 128],
                                c_sb[b // 4][:, b % 4, t, :],
                                start=(t == 0), stop=(t == 7))
                    nc.scalar.copy(vT[b // 4][:, :, b % 4, :], vT_ps[:])

            def pre_piece(grp, m):
                # capsules n = 4T+g for T in {2m, 2m+1}; fresh PSUM tile per piece
                pp = psPre.tile([128, 2, DC], f32, tag="pre")
                for tl in range(2):
                    for g in range(4):
                        n = 4 * (2 * m + tl) + g
                        for j in range(4):
                            nc.tensor.matmul(
                                pp[32 * g:32 * g + 4, tl],
                                vT[grp][:, j, :, n],
                                W16[:, j, 64 * n:64 * n + 64],
                                start=(j == 0), stop=(j == 3),
                                tile_position=(0, 32 * g),
                            )
                return pp

            def squash_piece(pp, dst, dsl):
                # dst[:, dsl] = pp / sqrt(|pp|^2 + eps), norm over d per capsule
                pre_c = sbP.tile([128, 2, DC], f32, tag="pre_c")
                sq2 = sbP.tile([128, 2, DC], f32, tag="sq2")
                nrm = sbP.tile([128, 2], f32, tag="nrm")
                srt = sbP.tile([128, 2], f32, tag="srt")
                scl = sbP.tile([128, 2], f32, tag="scl")
                nc.vector.tensor_copy(pre_c[:], pp[:])
                nc.vector.tensor_mul(sq2[:], pre_c[:], pp[:])
                nc.vector.tensor_reduce(nrm[:], sq2[:], AX, ADD)
                nc.scalar.activation(srt[:], nrm[:], AF.Sqrt, bias=eps_t[:])
                nc.vector.reciprocal(scl[:], srt[:])
                nc.vector.tensor_mul(dst[:, dsl], pp[:],
                                     scl[:].broadcast_to((128, 2, DC)))

            def w2_piece(grp, w2pn, m, o16):
                # transpose scaled outputs T-pair m -> outT[(tau d), (g c)],
                # mask into L, then contract d for this piece's capsule pairs
                tp = psT.tile([128, 128], f16, tag="tp")
                nc.tensor.transpose(
                    tp[:], o16[:].rearrange("p a b -> p (a b)"), IDENT[:])
                nc.vector.tensor_copy(outT[grp][:, m], tp[:])
                for tau in range(2):
                    nc.vector.tensor_copy(
                        L_sb[grp][m][64 * tau:64 * tau + 64, :, tau, :],
                        outT[grp][64 * tau:64 * tau + 64, m, :]
                        .rearrange("p (g c) -> p g c", g=4)[:, :, 0:4])
                for p in range(4 * m, 4 * m + 4):
                    for j in range(4):
                        nc.tensor.matmul(
                            w2pn[:, j, p], WT16[:, p, 128 * j:128 * j + 128],
                            L_sb[grp][m][:, p - 4 * m], start=True, stop=True)

            def w2_gather(grp, w2pn):
                w2v = w2T[grp][:].rearrange(
                    "p j b (m x g) -> p x j m g b", m=4, x=2, g=4)
                for tau in range(2):
                    for j in range(4):
                        nc.vector.tensor_copy(
                            w2v[:, tau, j],
                            w2pn[:, j, :, tau].rearrange(
                                "p (m g) b -> p m g b", g=4))

            def pre1_squash_w2(grp):
                # pre -> squash -> w2 for a 4-batch group, pipelined by T-pair
                with nc.named_scope(f"g{grp}_pre1w2"):
                    w2pn = psW2.tile([128, 4, 16, 2, 4], f32, tag="w2pn")
                    o16 = []
                    for m in range(4):
                        pp = pre_piece(grp, m)
                        o = sbO.tile([128, 2, DC], f16, tag="o16")
                        squash_piece(pp, o, slice(0, 2))
                        o16.append(o)
                        if m >= 1:
                            w2_piece(grp, w2pn, m - 1, o16[m - 1])
                    w2_piece(grp, w2pn, 3, o16[3])
                    w2_gather(grp, w2pn)

            def pre2_out(grp):
                # final pre + squash for one group, then its output DMA
                with nc.named_scope(f"pre2_out{grp}"):
                    for m in range(4):
                        pp = pre_piece(grp, m)
                        squash_piece(pp, outp32[grp], slice(2 * m, 2 * m + 2))
                    nc.sync.dma_start(
                        out_drams[grp][:],
                        outp32[grp][:].rearrange("p a b -> p (a b)"))

            # ================= schedule =================
            def pair1(a, b):
                bup(a, 1); bup(b, 1); vmm(a, 1); vmm(b, 1)

            def pair2(a, b):
                bup(a, 2); bup(b, 2); vmm(a, 2); vmm(b, 2)

            pair1(0, 1)
            pair1(2, 3)
            preload(AF.Sqrt, "sq0")
            pre1_squash_w2(0)
            preload(AF.Exp, "ex0")
            pair1(4, 5)
            pair2(0, 1)
            pair1(6, 7)
            pair2(2, 3)
            preload(AF.Sqrt, "sq1")
            pre1_squash_w2(1)
            pre2_out(0)
            preload(AF.Exp, "ex1")
            pair2(4, 5)
            pair2(6, 7)
            preload(AF.Sqrt, "sq2")
            pre2_out(1)

    nc.compile()
    return nc


def _host_prep(u_vecs, W):
    u_vecs = np.asarray(u_vecs, dtype=np.float32)
    W = np.asarray(W, dtype=np.float32).reshape(D_IN, NC_CAP * DC)
    Wr = W.reshape(D_IN, NC_CAP, DC)

    w16 = np.ascontiguousarray(
        W.reshape(4, 128, NC_CAP * DC).transpose(1, 0, 2)).astype(np.float16)
    # WT packed: [128=(tau,d), 16=(m,g), 512]; capsule n = 8m + 4tau + g
    wt = np.zeros((128, 16, D_IN), dtype=np.float16)
    for m in range(4):
        for g in range(4):
            for tau in range(2):
                n = 8 * m + 4 * tau + g
                wt[64 * tau:64 * tau + 64, 4 * m + g, :] = \
                    Wr[:, n, :].T.astype(np.float16)
    ident = np.eye(128, dtype=np.float16)

    in_maps = []
    for c in range(N_CORES):
        ub = u_vecs[c * B_LOC:(c + 1) * B_LOC]  # [8, 1024, 512] fp32
        u16 = ub.astype(np.float16)
        up = np.ascontiguousarray(
            u16.reshape(B_LOC, 8, 128, D_IN).transpose(0, 2, 1, 3))
        utp = np.ascontiguousarray(
            u16.transpose(0, 2, 1).reshape(B_LOC, 4, 128, N_IN)
            .transpose(0, 2, 1, 3))
        # host iter-0: c is uniform, so outputs0 depends only on column sums
        s = ub.sum(axis=1) / NC_CAP                       # [8, 512] fp32
        pre0 = np.einsum('bk,knd->bnd', s, Wr)
        out0 = pre0 / np.sqrt((pre0 ** 2).sum(-1, keepdims=True) + EPS)
        w20 = np.einsum('bnd,knd->bnk', out0, Wr)         # [8, 32, 512]
        w20t = np.ascontiguousarray(
            w20.transpose(2, 0, 1).reshape(4, 128, B_LOC, NC_CAP)
            .transpose(1, 0, 2, 3)).astype(np.float16)
        in_maps.append({
            "u16": up, "ut16": utp, "w16": w16, "wt16": wt, "w20t": w20t,
            "ident": ident,
        })
    return in_maps


def _unpack_out(rawA, rawB):
    # raw [128, 512] f32; row 32g+c, cols (T, d) -> out[4*grp + c, 4T+g, d]
    out = np.empty((B_LOC, NC_CAP, DC), dtype=np.float32)
    for grp, raw in enumerate((rawA, rawB)):
        r = raw.reshape(4, 32, 8, DC)   # [g, c-slot, T, d]
        for g in range(4):
            for cc in range(4):
                out[4 * grp + cc, 4 * np.arange(8) + g, :] = r[g, cc]
    return out


def kernel(u_vecs, W):
    from concourse.bass_utils import run_bass_kernel_spmd

    if "nc" not in _cached:
        _cached["nc"] = _build_program()
    nc = _cached["nc"]

    in_maps = _host_prep(u_vecs, W)
    res = run_bass_kernel_spmd(nc, in_maps, list(range(N_CORES)))
    out = np.concatenate(
        [_unpack_out(res.results[c]["outA"], res.results[c]["outB"])
         for c in range(N_CORES)], axis=0)
    return out.astype(np.float32)


# revision 24
# speedup vs baseline: 1.0780x; 1.0164x over previous
"""CapsNet dynamic-routing kernel for TRN2, 8 NeuronCores, data-parallel over batch.

Routing math is fully batch-local, so the kernel is a per-batch pipeline hidden
under the u-vec DMA stream:

  host: iter-0 (softmax(0) is uniform) -> outputs0, w20 = W @ outputs0 shipped
  chip: per batch b:  b1 = w20 @ u^T -> softmax -> v1 = c1^T u   (as UT/U land)
        per group:    pre1 = v1 @ W -> squash -> w21 = W @ out1  (T-pair pipelined)
        per batch:    b2 = w21 @ u^T -> softmax -> v2
        per group:    pre2 -> squash -> output DMA

All tiles that different pipeline stages touch are separate (per group /
per T-pair piece) so the tile framework's WAR tracking never serializes
independent stages. Scalar runs only Exp/Sqrt; table swaps are prefetched
with dummy activations at each boundary (the act table holds one function).
fp16 operands / fp32 accumulation; inputs host-packed partition-major.
"""

import numpy as np

ROUTINGS = 3
NC_CAP = 32
DC = 64
EPS = 1e-7
N_CORES = 8
B, N_IN, D_IN = 64, 1024, 512
B_LOC = B // N_CORES  # 8

_cached = {}


def _build_program():
    import concourse.bass as bass
    import concourse.tile as tile
    from concourse import bacc, mybir

    f16 = mybir.dt.float16
    f32 = mybir.dt.float32
    ADD = mybir.AluOpType.add
    AX = mybir.AxisListType.X
    AF = mybir.ActivationFunctionType

    nc = bacc.Bacc("TRN2", target_bir_lowering=False, debug=False,
                   num_devices=N_CORES)

    # host-packed, SBUF-native layouts (partition dim first, contiguous rows)
    w16_d = nc.dram_tensor("w16", [128, 4, NC_CAP * DC], f16, kind="ExternalInput").ap()
    wt16_d = nc.dram_tensor("wt16", [128, 16, D_IN], f16, kind="ExternalInput").ap()
    w20t_d = nc.dram_tensor("w20t", [128, 4, B_LOC, NC_CAP], f16, kind="ExternalInput").ap()
    ut_d = nc.dram_tensor("ut16", [B_LOC, 128, 4, N_IN], f16, kind="ExternalInput").ap()
    u_d = nc.dram_tensor("u16", [B_LOC, 128, 8, D_IN], f16, kind="ExternalInput").ap()
    ident_d = nc.dram_tensor("ident", [128, 128], f16, kind="ExternalInput").ap()
    outA_d = nc.dram_tensor("outA", [128, 8 * DC], f32, kind="ExternalOutput").ap()
    outB_d = nc.dram_tensor("outB", [128, 8 * DC], f32, kind="ExternalOutput").ap()
    out_drams = [outA_d, outB_d]

    with tile.TileContext(nc) as tc:
        with (
            tc.tile_pool(name="big", bufs=1) as big,
            tc.tile_pool(name="work", bufs=1) as work,
            tc.tile_pool(name="sbE", bufs=2) as sbE,
            tc.tile_pool(name="sbP", bufs=2) as sbP,
            tc.tile_pool(name="sbO", bufs=4) as sbO,
            tc.tile_pool(name="psB", bufs=2, space="PSUM") as psB,
            tc.tile_pool(name="psV", bufs=2, space="PSUM") as psV,
            tc.tile_pool(name="psPre", bufs=2, space="PSUM") as psPre,
            tc.tile_pool(name="psT", bufs=1, space="PSUM") as psT,
            tc.tile_pool(name="psW2", bufs=1, space="PSUM") as psW2,
        ):
            U = big.tile([128, B_LOC, 8, D_IN], f16, tag="U")      # (i%128),(b),(i//128),(k)
            UT = big.tile([128, B_LOC, 4, N_IN], f16, tag="UT")    # (k%128),(b),(k//128),(i)
            W16 = big.tile([128, 4, NC_CAP * DC], f16, tag="W16")  # (k%128),(k//128),(n d)
            WT16 = big.tile([128, 16, D_IN], f16, tag="WT16")      # (tau d),(m g),(k)
            W20T = big.tile([128, 4, B_LOC, NC_CAP], f16, tag="W20T")
            IDENT = work.tile([128, 128], f16, tag="IDENT")

            # per-group tiles (A: batches 0-3, B: 4-7) to avoid false WARs
            vT = [work.tile([128, 4, 4, NC_CAP], f16, tag=f"vT{g}",
                            name=f"vT{g}") for g in range(2)]
            w2T = [work.tile([128, 4, 4, NC_CAP], f16, tag=f"w2T{g}",
                             name=f"w2T{g}") for g in range(2)]
            c_sb = [work.tile([128, 4, 8, NC_CAP], f16, tag=f"c{g}",
                              name=f"c{g}") for g in range(2)]
            outT = [work.tile([128, 4, 128], f16, tag=f"outT{g}",
                              name=f"outT{g}") for g in range(2)]
            L_sb = [[work.tile([128, 4, 2, 4], f16, tag=f"L{g}_{m}",
                               name=f"L{g}_{m}") for m in range(4)]
                    for g in range(2)]
            outp32 = [work.tile([128, 8, DC], f32, tag=f"outp32_{g}",
                                name=f"outp32_{g}") for g in range(2)]
            z_sb = work.tile([128, B_LOC, 8], f32, tag="z")
            r_sb = work.tile([128, B_LOC, 8], f32, tag="r")
            eps_t = work.tile([128, 1], f32, tag="eps")
            dum = work.tile([128, 2], f32, tag="dum")

            # ---- DMA queue (single ring; ~8 concurrent HW channels)
            nc.sync.dma_start(IDENT[:], ident_d[:])
            nc.sync.dma_start(W20T[:], w20t_d[:])
            for b in range(2):
                nc.sync.dma_start(UT[:, b], ut_d[b])
                nc.sync.dma_start(U[:, b], u_d[b])
            nc.sync.dma_start(W16[:], w16_d[:])
            for b in range(2, 4):
                nc.sync.dma_start(UT[:, b], ut_d[b])
                nc.sync.dma_start(U[:, b], u_d[b])
            nc.sync.dma_start(WT16[:], wt16_d[:])
            for b in range(4, B_LOC):
                nc.sync.dma_start(UT[:, b], ut_d[b])
                nc.sync.dma_start(U[:, b], u_d[b])

            # ---- constants + Exp table warm (single-slot table: Exp only)
            nc.gpsimd.memset(eps_t[:], EPS)
            nc.gpsimd.memset(dum[:], 1.0)
            nc.scalar.activation(dum[:, 0:1], dum[:, 1:2], AF.Exp)
            for g in range(2):
                for m in range(4):
                    nc.gpsimd.memset(L_sb[g][m][:], 0.0)

            def preload(func, tag, dep):
                # dummy activation: prefetches the act table; dep anchors it
                # after the previous phase so the scheduler cannot hoist it
                with nc.named_scope(f"preload_{tag}"):
                    nc.scalar.activation(dum[:, 0:1], dep, func)

            def bup(b, it):
                # b-logits for batch b: [i%128, t, n] = sum_k u^T chunks @ w2T
                src = W20T if it == 1 else w2T[b // 4]
                bl = b if it == 1 else b % 4
                with nc.named_scope(f"i{it}_bup{b}"):
                    b_ps = psB.tile([128, 8, NC_CAP], f32, tag="b_ps")
                    for t in range(8):
                        for j in range(4):
                            nc.tensor.matmul(
                                b_ps[:, t], UT[:, b, j, 128 * t:128 * t + 128],
                                src[:, j, bl, :], start=(j == 0), stop=(j == 3))
                    e_sb = sbE.tile([128, 8, NC_CAP], f16, tag="e_sb")
                    nc.scalar.activation(e_sb[:], b_ps[:], AF.Exp)
                    nc.vector.tensor_reduce(z_sb[:, b], e_sb[:], AX, ADD)
                    nc.vector.reciprocal(r_sb[:, b], z_sb[:, b])
                    nc.vector.tensor_mul(
                        c_sb[b // 4][:, b % 4], e_sb[:],
                        r_sb[:, b].broadcast_to((128, 8, NC_CAP)))

            def vmm(b, it):
                with nc.named_scope(f"i{it}_v{b}"):
                    vT_ps = psV.tile([128, 4, NC_CAP], f32, tag="vT_ps")
                    for j in range(4):
                        for t in range(8):
                            nc.tensor.matmul(
                                vT_ps[:, j], U[:, b, t, 128 * j:128 * j + 128],
                                c_sb[b // 4][:, b % 4, t, :],
                                start=(t == 0), stop=(t == 7))
                    nc.scalar.copy(vT[b // 4][:, :, b % 4, :], vT_ps[:])

            def pre_piece(grp, m):
                # capsules n = 4T+g for T in {2m, 2m+1}; fresh PSUM tile per piece
                pp = psPre.tile([128, 2, DC], f32, tag="pre")
                for tl in range(2):
                    for g in range(4):
                        n = 4 * (2 * m + tl) + g
                        for j in range(4):
                            nc.tensor.matmul(
                                pp[32 * g:32 * g + 4, tl],
                                vT[grp][:, j, :, n],
                                W16[:, j, 64 * n:64 * n + 64],
                                start=(j == 0), stop=(j == 3),
                                tile_position=(0, 32 * g),
                            )
                return pp

            def squash_piece(pp, dst, dsl):
                # dst[:, dsl] = pp / sqrt(|pp|^2 + eps), norm over d per capsule
                pre_c = sbP.tile([128, 2, DC], f32, tag="pre_c")
                sq2 = sbP.tile([128, 2, DC], f32, tag="sq2")
                nrm = sbP.tile([128, 2], f32, tag="nrm")
                srt = sbP.tile([128, 2], f32, tag="srt")
                scl = sbP.tile([128, 2], f32, tag="scl")
                nc.scalar.copy(pre_c[:], pp[:])
                nc.vector.tensor_mul(sq2[:], pre_c[:], pp[:])
                nc.vector.tensor_reduce(nrm[:], sq2[:], AX, ADD)
                nc.scalar.activation(srt[:], nrm[:], AF.Sqrt, bias=eps_t[:])
                nc.vector.reciprocal(scl[:], srt[:])
                nc.vector.tensor_mul(dst[:, dsl], pp[:],
                                     scl[:].broadcast_to((128, 2, DC)))
                return scl

            def w2_piece(grp, w2pn, m, o16):
                # transpose scaled outputs T-pair m -> outT[(tau d), (g c)],
                # mask into L, then contract d for this piece's capsule pairs
                tp = psT.tile([128, 128], f16, tag="tp")
                nc.tensor.transpose(
                    tp[:], o16[:].rearrange("p a b -> p (a b)"), IDENT[:])
                nc.vector.tensor_copy(outT[grp][:, m], tp[:])
                for tau in range(2):
                    nc.vector.tensor_copy(
                        L_sb[grp][m][64 * tau:64 * tau + 64, :, tau, :],
                        outT[grp][64 * tau:64 * tau + 64, m, :]
                        .rearrange("p (g c) -> p g c", g=4)[:, :, 0:4])
                for p in range(4 * m, 4 * m + 4):
                    for j in range(4):
                        nc.tensor.matmul(
                            w2pn[:, j, p], WT16[:, p, 128 * j:128 * j + 128],
                            L_sb[grp][m][:, p - 4 * m], start=True, stop=True)

            def w2_gather(grp, w2pn):
                w2v = w2T[grp][:].rearrange(
                    "p j b (m x g) -> p x j m g b", m=4, x=2, g=4)
                for tau in range(2):
                    for j in range(4):
                        nc.scalar.copy(
                            w2v[:, tau, j],
                            w2pn[:, j, :, tau].rearrange(
                                "p (m g) b -> p m g b", g=4))

            def pre1_squash_w2(grp):
                # pre -> squash -> w2 for a 4-batch group, pipelined by T-pair
                with nc.named_scope(f"g{grp}_pre1w2"):
                    w2pn = psW2.tile([128, 4, 16, 2, 4], f32, tag="w2pn")
                    o16 = []
                    last_scl = None
                    for m in range(4):
                        pp = pre_piece(grp, m)
                        o = sbO.tile([128, 2, DC], f16, tag="o16")
                        last_scl = squash_piece(pp, o, slice(0, 2))
                        o16.append(o)
                        if m >= 1:
                            w2_piece(grp, w2pn, m - 1, o16[m - 1])
                    w2_piece(grp, w2pn, 3, o16[3])
                    w2_gather(grp, w2pn)
                    return last_scl

            def pre2_out(grp):
                # final pre + squash for one group, then its output DMA
                with nc.named_scope(f"pre2_out{grp}"):
                    last_scl = None
                    for m in range(4):
                        pp = pre_piece(grp, m)
                        last_scl = squash_piece(
                            pp, outp32[grp], slice(2 * m, 2 * m + 2))
                    nc.sync.dma_start(
                        out_drams[grp][:],
                        outp32[grp][:].rearrange("p a b -> p (a b)"))
                    return last_scl

            # ================= schedule =================
            def pair1(a, b):
                bup(a, 1); bup(b, 1); vmm(a, 1); vmm(b, 1)

            def pair2(a, b):
                bup(a, 2); bup(b, 2); vmm(a, 2); vmm(b, 2)

            pair1(0, 1)
            pair1(2, 3)
            preload(AF.Sqrt, "sq0", z_sb[:, 3, 0:1])
            sclA = pre1_squash_w2(0)
            preload(AF.Exp, "ex0", sclA[:, 0:1])
            pair1(4, 5)
            pair2(0, 1)
            pair1(6, 7)
            pair2(2, 3)
            preload(AF.Sqrt, "sq1", z_sb[:, 3, 0:1])
            sclB = pre1_squash_w2(1)
            scl2A = pre2_out(0)
            preload(AF.Exp, "ex1", sclB[:, 0:1])
            pair2(4, 5)
            pair2(6, 7)
            preload(AF.Sqrt, "sq2", z_sb[:, 7, 0:1])
            pre2_out(1)

    nc.compile()
    return nc


def _host_prep(u_vecs, W):
    u_vecs = np.asarray(u_vecs, dtype=np.float32)
    W = np.asarray(W, dtype=np.float32).reshape(D_IN, NC_CAP * DC)
    Wr = W.reshape(D_IN, NC_CAP, DC)

    w16 = np.ascontiguousarray(
        W.reshape(4, 128, NC_CAP * DC).transpose(1, 0, 2)).astype(np.float16)
    # WT packed: [128=(tau,d), 16=(m,g), 512]; capsule n = 8m + 4tau + g
    wt = np.zeros((128, 16, D_IN), dtype=np.float16)
    for m in range(4):
        for g in range(4):
            for tau in range(2):
                n = 8 * m + 4 * tau + g
                wt[64 * tau:64 * tau + 64, 4 * m + g, :] = \
                    Wr[:, n, :].T.astype(np.float16)
    ident = np.eye(128, dtype=np.float16)

    in_maps = []
    for c in range(N_CORES):
        ub = u_vecs[c * B_LOC:(c + 1) * B_LOC]  # [8, 1024, 512] fp32
        u16 = ub.astype(np.float16)
        up = np.ascontiguousarray(
            u16.reshape(B_LOC, 8, 128, D_IN).transpose(0, 2, 1, 3))
        utp = np.ascontiguousarray(
            u16.transpose(0, 2, 1).reshape(B_LOC, 4, 128, N_IN)
            .transpose(0, 2, 1, 3))
        # host iter-0: c is uniform, so outputs0 depends only on column sums
        s = ub.sum(axis=1) / NC_CAP                       # [8, 512] fp32
        pre0 = np.einsum('bk,knd->bnd', s, Wr)
        out0 = pre0 / np.sqrt((pre0 ** 2).sum(-1, keepdims=True) + EPS)
        w20 = np.einsum('bnd,knd->bnk', out0, Wr)         # [8, 32, 512]
        w20t = np.ascontiguousarray(
            w20.transpose(2, 0, 1).reshape(4, 128, B_LOC, NC_CAP)
            .transpose(1, 0, 2, 3)).astype(np.float16)
        in_maps.append({
            "u16": up, "ut16": utp, "w16": w16, "wt16": wt, "w20t": w20t,
            "ident": ident,
        })
    return in_maps


def _unpack_out(rawA, rawB):
    # raw [128, 512] f32; row 32g+c, cols (T, d) -> out[4*grp + c, 4T+g, d]
    out = np.empty((B_LOC, NC_CAP, DC), dtype=np.float32)
    for grp, raw in enumerate((rawA, rawB)):
        r = raw.reshape(4, 32, 8, DC)   # [g, c-slot, T, d]
        for g in range(4):
            for cc in range(4):
                out[4 * grp + cc, 4 * np.arange(8) + g, :] = r[g, cc]
    return out


def kernel(u_vecs, W):
    from concourse.bass_utils import run_bass_kernel_spmd

    if "nc" not in _cached:
        _cached["nc"] = _build_program()
    nc = _cached["nc"]

    in_maps = _host_prep(u_vecs, W)
    res = run_bass_kernel_spmd(nc, in_maps, list(range(N_CORES)))
    out = np.concatenate(
        [_unpack_out(res.results[c]["outA"], res.results[c]["outB"])
         for c in range(N_CORES)], axis=0)
    return out.astype(np.float32)
